# revision 2
# baseline (speedup 1.0000x reference)
"""Trainium2 Bass kernel for nn_BatchedQNodeLayer (8-qubit batched QNode).

Math: for an RX-angle-embedded product state pushed through a fixed
(theta-dependent) 2-layer strongly-entangling circuit and measured with
<Z_0>, the output is

    out_b = 0.5 + 0.5 * <psi(x_b)| M(theta) |psi(x_b)>

M expanded in the {I,Y,Z}^8 Pauli basis (X terms vanish for RX product
states) gives out_b as a multilinear form in per-wire features
[1, -sin(x_w), cos(x_w)].  The coefficient tensor factors hierarchically
(operator-Schmidt ranks are tiny for a shallow circuit; K=R1=R2=2 here),
reducing the per-element device work to ~130 elementwise MACs over
sin/cos planes.  All coefficients are computed on the host from theta
(O(1) in batch) and baked into the instruction stream as immediates.

Layout per core: batch shard of 16384 elements as [128 partitions, 128
free] planes; sin/cos via the ACT engine (range-reduced to [-pi, pi]
with the fp32 magic-rounding trick since the Sin table is only accurate
there); products/MAC-chains on the vector engine.
"""

import sys

sys.path.insert(0, "/opt/trn_rl_repo")

import numpy as np

N_QUBITS = 8
DIM = 256
N_CORES = 8
B_TOTAL = 131072
B_CORE = B_TOTAL // N_CORES  # 16384
P = 128                      # partitions
J = B_CORE // P              # 128 free elems per partition

TWO_PI = float(2.0 * np.pi)
INV_2PI = float(1.0 / (2.0 * np.pi))
MAGIC = float(1.5 * 2**23)   # fp32 round-to-nearest-integer bias
HALF_PI = float(np.pi / 2.0)


# ----------------------------------------------------------------------------
# Host-side precompute: theta -> hierarchical factor tensors
# ----------------------------------------------------------------------------

def _evolved_observable(theta):
    """M = U^dag Z0 U as dense 256x256 complex128 (numpy only)."""
    def rot(phi, th, om):
        c, s = np.cos(th / 2), np.sin(th / 2)
        return np.array([
            [np.exp(-0.5j * (phi + om)) * c, -np.exp(0.5j * (phi - om)) * s],
            [np.exp(-0.5j * (phi - om)) * s, np.exp(0.5j * (phi + om)) * c]])

    U = np.eye(DIM, dtype=np.complex128)

    def apply_1q(U, g, w):
        Ur = U.reshape([2] * N_QUBITS + [DIM])
        Ur = np.moveaxis(Ur, w, 0)
        Ur = np.tensordot(g, Ur, axes=([1], [0]))
        Ur = np.moveaxis(Ur, 0, w)
        return Ur.reshape(DIM, DIM)

    def apply_cnot(U, c, t):
        rows = np.arange(DIM)
        cbit = (rows >> (N_QUBITS - 1 - c)) & 1
        perm = np.where(cbit == 1, rows ^ (1 << (N_QUBITS - 1 - t)), rows)
        return U[perm, :]

    for l in range(2):
        for w in range(N_QUBITS):
            U = apply_1q(U, rot(*theta[l, w]), w)
        r = (l % (N_QUBITS - 1)) + 1
        for w in range(N_QUBITS):
            U = apply_cnot(U, w, (w + r) % N_QUBITS)
    z0 = 1.0 - 2.0 * ((np.arange(DIM) >> (N_QUBITS - 1)) & 1)
    return U.conj().T @ (z0[:, None] * U)


def _iyz_tensor(M):
    """Pauli coefficients over {I,Y,Z}^8 (axis order I,Y,Z per wire)."""
    I2 = np.eye(2, dtype=np.complex128)
    X = np.array([[0, 1], [1, 0]], dtype=np.complex128)
    Y = np.array([[0, -1j], [1j, 0]], dtype=np.complex128)
    Z = np.array([[1, 0], [0, -1]], dtype=np.complex128)
    T = M.reshape([2] * 16)
    perm = []
    for w in range(N_QUBITS):
        perm += [w, 8 + w]
    T = np.transpose(T, perm).reshape([4] * N_QUBITS)
    A = np.zeros((4, 4), dtype=np.complex128)
    for p, Pm in enumerate([I2, X, Y, Z]):
        A[p] = (Pm.T / 2).reshape(-1)
    for w in range(N_QUBITS):
        T = np.moveaxis(np.tensordot(A, T, axes=([1], [w])), 0, w)
    C = T.real
    idx = [0, 2, 3]
    return C[np.ix_(idx, idx, idx, idx, idx, idx, idx, idx)].copy()


def _factorize(theta, tol=1e-9):
    M = _evolved_observable(np.asarray(theta, np.float64))
    C = _iyz_tensor(M) * 0.5  # folds out = 0.5 + 0.5*ev
    S = C.reshape(81, 81)
    U, s, Vt = np.linalg.svd(S)
    K = max(1, int((s > s[0] * tol).sum()))
    A = U[:, :K] * np.sqrt(s[:K])
    Bv = Vt[:K].T * np.sqrt(s[:K])
    AL = A.reshape(9, 9, K)
    M1 = AL.reshape(9, 9 * K)
    P1, t1, Q1t = np.linalg.svd(M1, full_matrices=False)
    R1 = max(1, int((t1 > t1[0] * tol).sum()))
    W01 = P1[:, :R1] * np.sqrt(t1[:R1])                                  # [9,R1]
    V23 = Q1t[:R1].reshape(R1, 9, K) * np.sqrt(t1[:R1])[:, None, None]   # [R1,9,K]
    BR = Bv.reshape(9, 9, K).transpose(1, 0, 2)
    M2 = BR.reshape(9, 9 * K)
    P2, t2, Q2t = np.linalg.svd(M2, full_matrices=False)
    R2 = max(1, int((t2 > t2[0] * tol).sum()))
    W67 = P2[:, :R2] * np.sqrt(t2[:R2])                                  # [9,R2]
    V45 = Q2t[:R2].reshape(R2, 9, K) * np.sqrt(t2[:R2])[:, None, None]   # [R2,9,K]
    return dict(K=K, R1=R1, R2=R2, W01=W01, V23=V23, W67=W67, V45=V45)


# ----------------------------------------------------------------------------
# Bass program
# ----------------------------------------------------------------------------

def _build_program(F):
    from concourse import bass, mybir, tile
    from concourse.vector_clock import ScopedClock

    class SafeTileContext(tile.TileContext):
        """This walrus rejects instructions carrying more than one sync
        wait.  After scheduling, park every extra wait on a same-engine
        nop inserted immediately before the instruction."""

        def schedule_and_allocate(self):
            ret = super().schedule_and_allocate()
            nc = self.nc
            for bb in list(nc.main_func.blocks):
                i = 0
                while i < len(bb.instructions):
                    ins = bb.instructions[i]
                    si = ins.sync_info
                    waits = list(si.on_wait or []) if si else []
                    if len(waits) > 1:
                        ins.sync_info = mybir.SyncInfo(
                            on_wait=waits[:1], on_update=si.on_update)
                        nops = []
                        for w in waits[1:]:
                            n = nc.engines[ins.engine].nop()
                            n.ins.sync_info = mybir.SyncInfo(
                                on_wait=[w], on_update=[])
                            nops.append(n.ins)
                        for n in nops:
                            for blk in nc.main_func.blocks:
                                if n in blk.instructions:
                                    blk.instructions.remove(n)
                                    break
                        bb.instructions[i:i] = nops
                        i += len(nops)
                    i += 1
            return ret

    f32 = mybir.dt.float32
    OP = mybir.AluOpType
    AF = mybir.ActivationFunctionType

    nc = bass.Bass()
    x_in = nc.dram_tensor("x", [B_CORE, N_QUBITS], f32, kind="ExternalInput")
    y_out = nc.dram_tensor("out", [B_CORE, 1], f32, kind="ExternalOutput")

    with SafeTileContext(nc) as tc:
        with tc.tile_pool(name="pool", bufs=1) as pool:
            X = pool.tile([P, J * N_QUBITS], f32)        # (p, j*8+w)
            T1 = pool.tile([P, J * N_QUBITS], f32)
            Y = pool.tile([P, N_QUBITS * J], f32)        # w-major (p, w*128+j)
            SIN = pool.tile([P, N_QUBITS * J], f32)
            AB = pool.tile([P, N_QUBITS * J], f32)
            COS = pool.tile([P, N_QUBITS * J], f32)
            hp = pool.tile([P, 1], f32)

            nc.vector.memset(hp[:, :], HALF_PI)

            # input DMA, 8 queue-parallel chunks of 16 partitions
            xv = x_in.rearrange("(p j) w -> p (j w)", p=P)
            for c in range(8):
                nc.gpsimd.dma_start(X[16 * c:16 * (c + 1), :],
                                    xv[16 * c:16 * (c + 1), :])

            # range reduction: y = x - 2pi*round(x/(2pi)), reordered w-major
            nc.vector.tensor_scalar(T1[:, :], X[:, :], INV_2PI, MAGIC,
                                    OP.mult, OP.add)
            nc.vector.tensor_scalar(T1[:, :], T1[:, :], MAGIC, None,
                                    OP.subtract)
            # out free iter (w, j): Y[:, w*J+j] = T1[:, j*8+w]*(-2pi) + X[:, j*8+w]
            Yv = Y[:, :].rearrange("p (w j) -> p w j", w=N_QUBITS)
            T1v = T1[:, :].rearrange("p (j w) -> p w j", w=N_QUBITS)
            Xv = X[:, :].rearrange("p (j w) -> p w j", w=N_QUBITS)
            nc.vector.scalar_tensor_tensor(Yv, T1v, -TWO_PI, Xv,
                                           OP.mult, OP.add)

            # trig: sin(y); cos(y) = sin(pi/2 - |y|)
            nc.scalar.activation(SIN[:, :], Y[:, :], AF.Sin)
            nc.scalar.activation(AB[:, :], Y[:, :], AF.Abs)
            nc.scalar.activation(COS[:, :], AB[:, :], AF.Sin,
                                 bias=hp[:, :], scale=-1.0)

            def Sw(w):
                return SIN[:, w * J:(w + 1) * J]

            def Cw(w):
                return COS[:, w * J:(w + 1) * J]

            # pair products: for pair (wA,wB): PSS, PSC, PCS, PCC
            pairs = [(0, 1), (2, 3), (4, 5), (6, 7)]
            prod = {}
            for (wA, wB) in pairs:
                pss = pool.tile([P, J], f32, tag=f"pss{wA}")
                psc = pool.tile([P, J], f32, tag=f"psc{wA}")
                pcs = pool.tile([P, J], f32, tag=f"pcs{wA}")
                pcc = pool.tile([P, J], f32, tag=f"pcc{wA}")
                nc.vector.tensor_mul(pss[:, :], Sw(wA), Sw(wB))
                nc.vector.tensor_mul(psc[:, :], Sw(wA), Cw(wB))
                nc.vector.tensor_mul(pcs[:, :], Cw(wA), Sw(wB))
                nc.vector.tensor_mul(pcc[:, :], Cw(wA), Cw(wB))
                prod[(wA, wB)] = (pss, psc, pcs, pcc)

            def chain(name, pair, w9):
                """q = sum_a w9[a] * mono_a over pair; returns the q tile.

                mono index a = 3*iA+iB with per-wire features [1,-s,c]."""
                wA, wB = pair
                pss, psc, pcs, pcc = prod[pair]
                # (plane_ap, signed coefficient)
                terms = [
                    (Sw(wB), -w9[1]), (Cw(wB), w9[2]),
                    (Sw(wA), -w9[3]), (Cw(wA), w9[6]),
                    (pss[:, :], w9[4]), (psc[:, :], -w9[5]),
                    (pcs[:, :], -w9[7]), (pcc[:, :], w9[8]),
                ]
                terms = [(ap, c) for (ap, c) in terms if abs(c) > 1e-12]
                q = pool.tile([P, J], f32, tag=name)
                if not terms:
                    nc.vector.memset(q[:, :], float(w9[0]))
                    return q
                ap0, c0 = terms[0]
                nc.vector.tensor_scalar(q[:, :], ap0, float(c0), float(w9[0]),
                                        OP.mult, OP.add)
                for (ap, c) in terms[1:]:
                    nc.vector.scalar_tensor_tensor(q[:, :], ap, float(c),
                                                   q[:, :], OP.mult, OP.add)
                return q

            K, R1, R2 = F["K"], F["R1"], F["R2"]
            u01 = [chain(f"u01_{m}", (0, 1), F["W01"][:, m]) for m in range(R1)]
            u67 = [chain(f"u67_{m}", (6, 7), F["W67"][:, m]) for m in range(R2)]
            v23 = [[chain(f"v23_{m}_{k}", (2, 3), F["V23"][m, :, k])
                    for k in range(K)] for m in range(R1)]
            v45 = [[chain(f"v45_{m}_{k}", (4, 5), F["V45"][m, :, k])
                    for k in range(K)] for m in range(R2)]

            def combine(name, us, vs):
                """sum_m us[m]*vs[m]"""
                acc = pool.tile([P, J], f32, tag=name)
                nc.vector.tensor_mul(acc[:, :], us[0][:, :], vs[0][:, :])
                for m in range(1, len(us)):
                    t = pool.tile([P, J], f32, tag=name + "t")
                    nc.vector.tensor_mul(t[:, :], us[m][:, :], vs[m][:, :])
                    nc.vector.tensor_add(acc[:, :], acc[:, :], t[:, :])
                return acc

            uL = [combine(f"uL{k}", u01, [v23[m][k] for m in range(R1)])
                  for k in range(K)]
            uR = [combine(f"uR{k}", u67, [v45[m][k] for m in range(R2)])
                  for k in range(K)]

            acc = pool.tile([P, J], f32)
            nc.vector.tensor_mul(acc[:, :], uL[0][:, :], uR[0][:, :])
            for k in range(1, K):
                t = pool.tile([P, J], f32, tag="topt")
                nc.vector.tensor_mul(t[:, :], uL[k][:, :], uR[k][:, :])
                nc.vector.tensor_add(acc[:, :], acc[:, :], t[:, :])
            OUT = pool.tile([P, J], f32)
            nc.vector.tensor_scalar(OUT[:, :], acc[:, :], 1.0, 0.5,
                                    OP.mult, OP.add)

            yv = y_out.rearrange("(p j) o -> p (j o)", p=P)
            for c in range(4):
                nc.gpsimd.dma_start(yv[32 * c:32 * (c + 1), :],
                                    OUT[32 * c:32 * (c + 1), :])
    return nc


_PROGRAM_CACHE = {}
LAST_RESULT = None


def kernel(x: np.ndarray, theta: np.ndarray) -> np.ndarray:
    import os
    from concourse.bass_utils import run_bass_kernel_spmd

    x = np.ascontiguousarray(np.asarray(x, dtype=np.float32))
    theta = np.asarray(theta, dtype=np.float32)
    assert x.shape == (B_TOTAL, N_QUBITS), x.shape

    key = theta.tobytes()
    nc = _PROGRAM_CACHE.get(key)
    if nc is None:
        F = _factorize(theta)
        nc = _build_program(F)
        _PROGRAM_CACHE[key] = nc

    shards = [x[i * B_CORE:(i + 1) * B_CORE] for i in range(N_CORES)]
    in_maps = [{"x": s} for s in shards]
    trace = bool(int(os.environ.get("KERNEL_PROFILE", "0")))
    res = run_bass_kernel_spmd(nc, in_maps, list(range(N_CORES)), trace=trace)
    global LAST_RESULT
    LAST_RESULT = res
    out = np.concatenate([res.results[i]["out"] for i in range(N_CORES)], axis=0)
    return out.astype(np.float32, copy=False)
